# revision 3
# baseline (speedup 1.0000x reference)
"""APPNP GNN forward on 8 Trainium2 NeuronCores (Bass/Tile, SPMD).

Strategy (all 8 cores share one instruction stream; per-core data differs):
  - nodes sharded 12500/core; MLP data-parallel; z table fp16 [100000,128]
    (cols 47..127 zero) replicated in HBM via AllGather each step
  - edges partitioned by dst core, sorted by (dst chunk of 128, src block of
    25000, dst); 128-slot tiles gathered via dma_gather (int16 block-local
    indices, 256B rows, single_packet=False)
  - per tile, a [128, wdt] fp16 S matrix (0.9*gcn_norm weights at
    [slot, dst_col]) aggregates via TensorE into a per-chunk PSUM [128, 64]
  - epilogue adds alpha*h, casts fp16 to staging; AllGather -> next z table
  - final iteration computes log_softmax on-chip, fp32 out [12500, 47]
"""
import sys, os, types

sys.path.insert(0, "/opt/trn_rl_repo")
import numpy as np

N = 100000
NCORE = 8
NPC = N // NCORE
CH = 128
NCHK = (NPC + CH - 1) // CH  # 98
BLOCKS = 4
BLK = N // BLOCKS
GSIZE = 8
ALPHA = 0.1
MAX_CALL_TILES = 32
K_ITERS = 10
F_IN = 128
HID = 256
CLS = 47

TRACE = False           # set by test harness for NTFF profiling
LAST_EXEC_NS = None
LAST_SCOPES = None


def _chunk_size(i):
    return min(CH, NPC - CH * i)


def _preprocess(edge_index):
    src = np.asarray(edge_index[0], dtype=np.int64)
    dst = np.asarray(edge_index[1], dtype=np.int64)

    deg = np.bincount(dst, minlength=N).astype(np.float64) + 1.0
    dinv = 1.0 / np.sqrt(deg)
    ew = (dinv[src] * dinv[dst]) * (1.0 - ALPHA)
    self_w = (dinv * dinv) * (1.0 - ALPHA)

    all_src = np.concatenate([src, np.arange(N)])
    all_dst = np.concatenate([dst, np.arange(N)])
    all_w = np.concatenate([ew, self_w]).astype(np.float32)

    core = all_dst // NPC
    dloc = all_dst % NPC
    chunk = dloc // CH
    col = dloc % CH
    blk = all_src // BLK
    sloc = all_src % BLK

    order = np.lexsort((col, blk, chunk, core))
    core_s = core[order]; chunk_s = chunk[order]; blk_s = blk[order]
    col_s = col[order]; sloc_s = sloc[order]; w_s = all_w[order]

    key = ((core_s * NCHK) + chunk_s) * BLOCKS + blk_s
    nkeys = NCORE * NCHK * BLOCKS
    starts = np.searchsorted(key, np.arange(nkeys), side="left")
    ends = np.searchsorted(key, np.arange(nkeys), side="right")

    def run(c, i, b):
        k = (c * NCHK + i) * BLOCKS + b
        return (col_s[starts[k]:ends[k]], sloc_s[starts[k]:ends[k]],
                w_s[starts[k]:ends[k]])

    tiles_ib = {}
    pertile_ib = {}
    for i in range(NCHK):
        szi = _chunk_size(i)
        for b in range(BLOCKS):
            runs = [run(c, i, b) for c in range(NCORE)]
            ptrs = [0] * NCORE
            tlist, pertile = [], []
            while any(ptrs[c] < len(runs[c][0]) for c in range(NCORE)):
                pend = [runs[c][0][ptrs[c]] for c in range(NCORE)
                        if ptrs[c] < len(runs[c][0])]
                base = int(min(pend))
                # PSUM matmul outputs may start only at partition 0/32/64
                # with max widths 128/32/64: snap the window base down, and
                # cap tiles starting in [32,64) at col 63 so width <= 32.
                sb = min(32 * (base // 32), 64)
                colcap = 63 if sb == 32 else szi - 1
                hi = base
                while hi + 1 <= colcap:
                    nxt = hi + 1
                    if all(np.searchsorted(runs[c][0], nxt, side="right")
                           - ptrs[c] <= 128 for c in range(NCORE)):
                        hi = nxt
                    else:
                        break
                entry = []
                maxcol = base
                for c in range(NCORE):
                    cols, slocs, ws = runs[c]
                    e_ = int(np.searchsorted(cols, hi, side="right"))
                    e_ = min(e_, ptrs[c] + 128)
                    sl = slice(ptrs[c], e_)
                    entry.append((cols[sl], slocs[sl], ws[sl]))
                    if e_ > ptrs[c]:
                        maxcol = max(maxcol, int(cols[e_ - 1]))
                    ptrs[c] = e_
                tlist.append((sb, maxcol - sb + 1))
                pertile.append(entry)
            tiles_ib[(i, b)] = tlist
            pertile_ib[(i, b)] = pertile

    groups = []
    i = 0
    while i < NCHK:
        groups.append(list(range(i, min(i + GSIZE, NCHK))))
        i += GSIZE

    tile_info = []
    entries = []
    calls = []
    for gi, g in enumerate(groups):
        for b in range(BLOCKS):
            pend_tiles = []
            for i_ in g:
                for (base, wdt), entry in zip(tiles_ib[(i_, b)],
                                              pertile_ib[(i_, b)]):
                    pend_tiles.append((i_, base, wdt, entry))
            for cstart in range(0, len(pend_tiles), MAX_CALL_TILES):
                ct = pend_tiles[cstart:cstart + MAX_CALL_TILES]
                cid = len(calls)
                calls.append({"t0": len(tile_info), "nt": len(ct), "b": b,
                              "g": gi})
                for k, (i_, base, wdt, entry) in enumerate(ct):
                    tile_info.append({"i": i_, "b": b, "base": base,
                                      "wdt": wdt, "call": cid, "tloc": k})
                    entries.append(entry)

    NT = len(tile_info)
    SLOTS = NT * 128

    soff = 0
    group_s0 = [None] * len(groups)
    group_send = [0] * len(groups)
    for t, ti in enumerate(tile_info):
        gi = calls[ti["call"]]["g"]
        if group_s0[gi] is None:
            group_s0[gi] = soff
        ti["soff"] = soff
        soff += ti["wdt"]
        group_send[gi] = soff
    SUMW = soff

    idx16 = np.zeros((NCORE, SLOTS), np.int16)
    sdata = np.zeros((NCORE, 128, SUMW), np.float16)
    for t, (ti, entry) in enumerate(zip(tile_info, entries)):
        s0 = t * 128
        for c in range(NCORE):
            cols, slocs, ws = entry[c]
            n = len(cols)
            if n == 0:
                continue
            idx16[c, s0:s0 + n] = slocs.astype(np.int16)
            sdata[c, np.arange(n), ti["soff"] + (cols - ti["base"])] = (
                ws.astype(np.float16))

    idx_sb = np.zeros((NCORE, 128, SLOTS // 16), np.int16)
    off16 = 0
    for call in calls:
        call["idx_off16"] = off16
        nsl = call["nt"] * 128
        s0 = call["t0"] * 128
        for c in range(NCORE):
            seg = idx16[c, s0:s0 + nsl]
            idx_sb[c, :, off16:off16 + nsl // 16] = np.tile(
                seg.reshape(nsl // 16, 16).T, (8, 1))
        off16 += nsl // 16

    static = {"groups": groups, "tile_info": tile_info, "calls": calls,
              "NT": NT, "SLOTS": SLOTS, "SUMW": SUMW,
              "group_s0": group_s0, "group_send": group_send}
    return static, {"idx_sb": idx_sb, "sdata": sdata, "idx16": idx16}


def _install_ntff_hook():
    from concourse import bass_utils
    try:
        import antenv
        from trn_agent_boot.trn_boot import _ntff_profile_via_ctypes
    except Exception:
        return
    if "antenv.axon_hooks" in sys.modules:
        return
    mod = types.ModuleType("antenv.axon_hooks")
    state = {"hook": None}
    mod.set_axon_ntff_profile_hook = lambda h: state.__setitem__("hook", h)
    mod.get_axon_ntff_profile_hook = lambda: state["hook"]
    sys.modules["antenv.axon_hooks"] = mod
    antenv.axon_hooks = mod
    mod.set_axon_ntff_profile_hook(
        _ntff_profile_via_ctypes("/opt/axon/libaxon_pjrt.so"))
    bass_utils.upload_artifacts = lambda tmpdir: f"local:{tmpdir}"


def _build(static):
    import concourse.bass as bass
    import concourse.bacc as bacc
    import concourse.tile as tile
    import concourse.mybir as mybir
    from concourse.masks import make_identity

    f32 = mybir.dt.float32
    f16 = mybir.dt.float16
    i16 = mybir.dt.int16
    AF = mybir.ActivationFunctionType
    OP = mybir.AluOpType
    AX = mybir.AxisListType

    groups = static["groups"]
    tile_info = static["tile_info"]
    calls = static["calls"]
    SLOTS = static["SLOTS"]
    SUMW = static["SUMW"]
    group_s0 = static["group_s0"]
    group_send = static["group_send"]

    # per-chunk tile lists (slot order)
    chunk_tiles = {i: [] for i in range(NCHK)}
    for t, ti in enumerate(tile_info):
        chunk_tiles[ti["i"]].append(t)

    nc = bacc.Bacc("TRN2", target_bir_lowering=False, debug=False,
                   num_devices=NCORE, num_swdge_queues=4)

    x_d = nc.dram_tensor("x_sh", [NPC, F_IN], f32, kind="ExternalInput").ap()
    W1_d = nc.dram_tensor("w1", [F_IN, HID], f32, kind="ExternalInput").ap()
    W2_d = nc.dram_tensor("w2", [HID, CLS], f32, kind="ExternalInput").ap()
    b1_d = nc.dram_tensor("b1c", [128, 2], f32, kind="ExternalInput").ap()
    b2_d = nc.dram_tensor("b2r", [128, CLS], f32, kind="ExternalInput").ap()
    idx_d = nc.dram_tensor("idxs", [128, SLOTS // 16], i16,
                           kind="ExternalInput").ap()
    sd_d = nc.dram_tensor("sdata", [128, SUMW], f16, kind="ExternalInput").ap()
    out_d = nc.dram_tensor("out", [NPC, CLS], f32, kind="ExternalOutput").ap()

    agin = nc.dram_tensor("agin", [NPC, 128], f16).ap()
    ztab = nc.dram_tensor("ztab", [N, 128], f16, addr_space="Shared").ap()

    with tile.TileContext(nc) as tc:
        with (
            tc.tile_pool(name="const", bufs=1) as cp,
            tc.tile_pool(name="resident", bufs=1) as rp,
            tc.tile_pool(name="mlp", bufs=3) as mp,
            tc.tile_pool(name="gb", bufs=5) as gp,
            tc.tile_pool(name="st", bufs=2) as stp,
            tc.tile_pool(name="sm", bufs=3) as smp,
        ):
            # constants / residents
            idx_t = rp.tile([128, SLOTS // 16], i16)
            nc.sync.dma_start(idx_t[:], idx_d[:])
            W1_t = cp.tile([128, HID], f32)
            nc.sync.dma_start(W1_t[:], W1_d[:])
            W2a_t = cp.tile([128, CLS], f32)
            nc.sync.dma_start(W2a_t[:], W2_d[0:128, :])
            W2b_t = cp.tile([128, CLS], f32)
            nc.sync.dma_start(W2b_t[:], W2_d[128:256, :])
            b1_t = cp.tile([128, 2], f32)
            nc.sync.dma_start(b1_t[:], b1_d[:])
            b2_t = cp.tile([128, CLS], f32)
            nc.sync.dma_start(b2_t[:], b2_d[:])
            ident = cp.tile([128, 128], f32)
            make_identity(nc, ident[:])
            zeroS = cp.tile([128, 128], f16)
            nc.vector.memset(zeroS[:], 0.0)
            ah_t = rp.tile([128, NCHK * 64], f32)
            nc.vector.memset(ah_t[:], 0.0)
            stg = rp.tile([128, NCHK * 128], f16)
            nc.vector.memset(stg[:], 0.0)

            # ---- MLP: z0 = relu(x@W1+b1)@W2+b2 ----
            with tc.tile_pool(name="psmlp", bufs=2, space="PSUM") as pmp:
                for i in range(NCHK):
                    sz = _chunk_size(i)
                    xt = mp.tile([128, F_IN], f32, tag="xt")
                    nc.sync.dma_start(xt[0:sz, :], x_d[CH * i:CH * i + sz, :])
                    pxT = pmp.tile([128, 128], f32, tag="pmlp")
                    nc.tensor.transpose(pxT[:, 0:sz], xt[0:sz, :],
                                        ident[0:sz, 0:sz])
                    xT = mp.tile([128, 128], f32, tag="xT")
                    nc.scalar.activation(xT[:, 0:sz], pxT[:, 0:sz], AF.Copy)
                    relus = []
                    for h in range(2):
                        ph = pmp.tile([128, 128], f32, tag="pmlp")
                        nc.tensor.matmul(ph[:, 0:sz],
                                         lhsT=W1_t[:, 128 * h:128 * (h + 1)],
                                         rhs=xT[:, 0:sz], start=True,
                                         stop=True)
                        rh = mp.tile([128, 128], f32, tag=f"relu{h}")
                        nc.scalar.activation(rh[:, 0:sz], ph[:, 0:sz],
                                             AF.Relu, bias=b1_t[:, h:h + 1])
                        relus.append(rh)
                    pz = pmp.tile([128, 128], f32, tag="pmlp")
                    for h in range(2):
                        nc.tensor.matmul(pz[0:sz, 0:CLS],
                                         lhsT=relus[h][:, 0:sz],
                                         rhs=(W2a_t if h == 0 else W2b_t)[:],
                                         start=(h == 0), stop=(h == 1))
                    z0 = mp.tile([128, CLS], f32, tag="z0")
                    nc.vector.tensor_tensor(out=z0[0:sz, :],
                                            in0=pz[0:sz, 0:CLS],
                                            in1=b2_t[0:sz, :], op=OP.add)
                    nc.vector.tensor_copy(
                        out=stg[0:sz, 128 * i:128 * i + CLS], in_=z0[0:sz, :])
                    nc.scalar.mul(ah_t[0:sz, 64 * i:64 * i + CLS],
                                  z0[0:sz, :], ALPHA)

            stg3 = stg[:].rearrange("p (i f) -> p i f", f=128)
            ag_dst1 = agin[0:(NCHK - 1) * CH, :].rearrange(
                "(i p) f -> p i f", p=128)

            def do_ag():
                nc.sync.dma_start(ag_dst1[:], stg3[:, 0:NCHK - 1, :])
                nc.sync.dma_start(agin[(NCHK - 1) * CH:NPC, :],
                                  stg[0:_chunk_size(NCHK - 1),
                                      128 * (NCHK - 1):128 * NCHK])
                nc.gpsimd.collective_compute(
                    "AllGather", mybir.AluOpType.bypass,
                    replica_groups=[list(range(NCORE))],
                    ins=[agin[:].opt()], outs=[ztab[:].opt()])

            do_ag()

            # ---- K propagation steps ----
            with tc.tile_pool(name="pschunk", bufs=8,
                              space="PSUM") as psp:
                calls_of_group = {}
                for cid, call in enumerate(calls):
                    calls_of_group.setdefault(call["g"], []).append(cid)

                max_sw = max(group_send[g] - group_s0[g]
                             for g in range(len(groups)))
                for k in range(1, K_ITERS + 1):
                    for gi, grp in enumerate(groups):
                        sw = group_send[gi] - group_s0[gi]
                        st_g = stp.tile([128, max_sw], f16, tag="stg")
                        nc.sync.dma_start(
                            st_g[:, 0:sw],
                            sd_d[:, group_s0[gi]:group_send[gi]])
                        gtile = {}
                        for qi, cid in enumerate(calls_of_group[gi]):
                            call = calls[cid]
                            nt = call["nt"]
                            b = call["b"]
                            g = gp.tile([128, MAX_CALL_TILES, 128], f16, tag="g")
                            nc.gpsimd.dma_gather(
                                g[:, 0:nt, :],
                                ztab[BLK * b:BLK * (b + 1), :],
                                idx_t[:, call["idx_off16"]:
                                      call["idx_off16"] + nt * 8],
                                nt * 128, nt * 128, 128,
                                single_packet=False,
                                queue_num=qi % 4,
                            )
                            gtile[cid] = g
                        for i in grp:
                            sz = _chunk_size(i)
                            ps = psp.tile([128, 64], f32, tag="ps")
                            nc.tensor.matmul(ps[:, :], lhsT=zeroS[:, 0:128],
                                             rhs=zeroS[:, 0:64],
                                             start=True, stop=False)
                            tl = chunk_tiles[i]
                            for j, t in enumerate(tl):
                                ti = tile_info[t]
                                loff = ti["soff"] - group_s0[gi]
                                nc.tensor.matmul(
                                    ps[ti["base"]:ti["base"] + ti["wdt"], 0:64],
                                    lhsT=st_g[:, loff:loff + ti["wdt"]],
                                    rhs=gtile[ti["call"]][:, ti["tloc"], 0:64],
                                    start=False, stop=(j == len(tl) - 1))
                            if k < K_ITERS:
                                nc.vector.tensor_tensor(
                                    out=stg[0:sz, 128 * i:128 * i + 64],
                                    in0=ps[0:sz, 0:64],
                                    in1=ah_t[0:sz, 64 * i:64 * (i + 1)],
                                    op=OP.add)
                            else:
                                zf = smp.tile([128, 64], f32, tag="zf")
                                nc.vector.tensor_tensor(
                                    out=zf[0:sz, :], in0=ps[0:sz, 0:64],
                                    in1=ah_t[0:sz, 64 * i:64 * (i + 1)],
                                    op=OP.add)
                                m = smp.tile([128, 1], f32, tag="m")
                                nc.vector.tensor_reduce(
                                    m[0:sz, :], zf[0:sz, 0:CLS], axis=AX.X,
                                    op=OP.max)
                                nm = smp.tile([128, 1], f32, tag="nm")
                                nc.vector.tensor_scalar_mul(
                                    nm[0:sz, :], m[0:sz, :], -1.0)
                                e = smp.tile([128, CLS], f32, tag="e")
                                nc.scalar.activation(e[0:sz, :], zf[0:sz, 0:CLS],
                                                     AF.Exp, bias=nm[0:sz, 0:1])
                                s = smp.tile([128, 1], f32, tag="s")
                                nc.vector.tensor_reduce(
                                    s[0:sz, :], e[0:sz, :], axis=AX.X, op=OP.add)
                                ls = smp.tile([128, 1], f32, tag="ls")
                                nc.scalar.activation(ls[0:sz, :], s[0:sz, :],
                                                     AF.Ln)
                                offs = smp.tile([128, 1], f32, tag="offs")
                                nc.vector.tensor_tensor(
                                    out=offs[0:sz, :], in0=m[0:sz, :],
                                    in1=ls[0:sz, :], op=OP.add)
                                res = smp.tile([128, CLS], f32, tag="res")
                                nc.vector.tensor_scalar(
                                    out=res[0:sz, :], in0=zf[0:sz, 0:CLS],
                                    scalar1=offs[0:sz, 0:1], scalar2=None,
                                    op0=OP.subtract)
                                nc.sync.dma_start(
                                    out_d[CH * i:CH * i + sz, :], res[0:sz, :])
                    if k < K_ITERS:
                        do_ag()

    nc.compile()
    return nc


_CACHE = {}


def kernel(x, edge_index, W1, b1, W2, b2):
    global LAST_EXEC_NS, LAST_SCOPES
    from concourse import bass_utils

    x = np.asarray(x, np.float32)
    ei = np.asarray(edge_index)
    W1 = np.asarray(W1, np.float32)
    b1 = np.asarray(b1, np.float32)
    W2 = np.asarray(W2, np.float32)
    b2 = np.asarray(b2, np.float32)

    static, per_core = _preprocess(ei)
    nc = _build(static)

    b1c = np.stack([b1[0:128], b1[128:256]], axis=1).astype(np.float32)
    b1c = np.ascontiguousarray(b1c)
    b2r = np.ascontiguousarray(np.tile(b2[None, :], (128, 1)).astype(np.float32))

    in_maps = []
    for c in range(NCORE):
        in_maps.append({
            "x_sh": np.ascontiguousarray(x[c * NPC:(c + 1) * NPC]),
            "w1": W1, "w2": W2, "b1c": b1c, "b2r": b2r,
            "idxs": np.ascontiguousarray(per_core["idx_sb"][c]),
            "sdata": np.ascontiguousarray(per_core["sdata"][c]),
        })

    if TRACE:
        _install_ntff_hook()
    res = bass_utils.run_bass_kernel_spmd(
        nc, in_maps, core_ids=list(range(NCORE)), trace=TRACE)
    LAST_EXEC_NS = res.exec_time_ns
    LAST_SCOPES = res.per_core_scope_times

    out = np.concatenate([res.results[c]["out"] for c in range(NCORE)], axis=0)
    return out.astype(np.float32)



# revision 5
# speedup vs baseline: 1.0156x; 1.0156x over previous
"""APPNP GNN forward on 8 Trainium2 NeuronCores (Bass/Tile, SPMD).

Strategy (all 8 cores share one instruction stream; per-core data differs):
  - nodes sharded 12500/core; MLP data-parallel; z table fp16 [100000,128]
    (cols 47..127 zero) replicated in HBM via AllGather each step
  - edges partitioned by dst core, sorted by (dst chunk of 128, src block of
    25000, dst); 128-slot tiles gathered via dma_gather (int16 block-local
    indices, 256B rows, single_packet=False)
  - per tile, a [128, wdt] fp16 S matrix (0.9*gcn_norm weights at
    [slot, dst_col]) aggregates via TensorE into a per-chunk PSUM [128, 64]
  - epilogue adds alpha*h, casts fp16 to staging; AllGather -> next z table
  - final iteration computes log_softmax on-chip, fp32 out [12500, 47]
"""
import sys, os, types

sys.path.insert(0, "/opt/trn_rl_repo")
import numpy as np

N = 100000
NCORE = 8
NPC = N // NCORE
CH = 128
NCHK = (NPC + CH - 1) // CH  # 98
BLOCKS = 4
BLK = N // BLOCKS
GSIZE = 8
ALPHA = 0.1
MAX_CALL_TILES = 8
K_ITERS = 10
F_IN = 128
HID = 256
CLS = 47

TRACE = False           # set by test harness for NTFF profiling
LAST_EXEC_NS = None
LAST_SCOPES = None


def _chunk_size(i):
    return min(CH, NPC - CH * i)


def _preprocess(edge_index):
    src = np.asarray(edge_index[0], dtype=np.int64)
    dst = np.asarray(edge_index[1], dtype=np.int64)

    deg = np.bincount(dst, minlength=N).astype(np.float64) + 1.0
    dinv = 1.0 / np.sqrt(deg)
    ew = (dinv[src] * dinv[dst]) * (1.0 - ALPHA)
    self_w = (dinv * dinv) * (1.0 - ALPHA)

    all_src = np.concatenate([src, np.arange(N)])
    all_dst = np.concatenate([dst, np.arange(N)])
    all_w = np.concatenate([ew, self_w]).astype(np.float32)

    core = all_dst // NPC
    dloc = all_dst % NPC
    chunk = dloc // CH
    col = dloc % CH
    blk = all_src // BLK
    sloc = all_src % BLK

    order = np.lexsort((col, blk, chunk, core))
    core_s = core[order]; chunk_s = chunk[order]; blk_s = blk[order]
    col_s = col[order]; sloc_s = sloc[order]; w_s = all_w[order]

    key = ((core_s * NCHK) + chunk_s) * BLOCKS + blk_s
    nkeys = NCORE * NCHK * BLOCKS
    starts = np.searchsorted(key, np.arange(nkeys), side="left")
    ends = np.searchsorted(key, np.arange(nkeys), side="right")

    def run(c, i, b):
        k = (c * NCHK + i) * BLOCKS + b
        return (col_s[starts[k]:ends[k]], sloc_s[starts[k]:ends[k]],
                w_s[starts[k]:ends[k]])

    tiles_ib = {}
    pertile_ib = {}
    for i in range(NCHK):
        szi = _chunk_size(i)
        for b in range(BLOCKS):
            runs = [run(c, i, b) for c in range(NCORE)]
            ptrs = [0] * NCORE
            tlist, pertile = [], []
            while any(ptrs[c] < len(runs[c][0]) for c in range(NCORE)):
                pend = [runs[c][0][ptrs[c]] for c in range(NCORE)
                        if ptrs[c] < len(runs[c][0])]
                base = int(min(pend))
                # PSUM matmul outputs may start only at partition 0/32/64
                # with max widths 128/32/64: snap the window base down, and
                # cap tiles starting in [32,64) at col 63 so width <= 32.
                sb = min(32 * (base // 32), 64)
                colcap = 63 if sb == 32 else szi - 1
                hi = base
                while hi + 1 <= colcap:
                    nxt = hi + 1
                    if all(np.searchsorted(runs[c][0], nxt, side="right")
                           - ptrs[c] <= 128 for c in range(NCORE)):
                        hi = nxt
                    else:
                        break
                entry = []
                maxcol = base
                for c in range(NCORE):
                    cols, slocs, ws = runs[c]
                    e_ = int(np.searchsorted(cols, hi, side="right"))
                    e_ = min(e_, ptrs[c] + 128)
                    sl = slice(ptrs[c], e_)
                    entry.append((cols[sl], slocs[sl], ws[sl]))
                    if e_ > ptrs[c]:
                        maxcol = max(maxcol, int(cols[e_ - 1]))
                    ptrs[c] = e_
                tlist.append((sb, maxcol - sb + 1))
                pertile.append(entry)
            tiles_ib[(i, b)] = tlist
            pertile_ib[(i, b)] = pertile

    groups = []
    i = 0
    while i < NCHK:
        groups.append(list(range(i, min(i + GSIZE, NCHK))))
        i += GSIZE

    tile_info = []
    entries = []
    calls = []
    for gi, g in enumerate(groups):
        for b in range(BLOCKS):
            pend_tiles = []
            for i_ in g:
                for (base, wdt), entry in zip(tiles_ib[(i_, b)],
                                              pertile_ib[(i_, b)]):
                    pend_tiles.append((i_, base, wdt, entry))
            for cstart in range(0, len(pend_tiles), MAX_CALL_TILES):
                ct = pend_tiles[cstart:cstart + MAX_CALL_TILES]
                cid = len(calls)
                calls.append({"t0": len(tile_info), "nt": len(ct), "b": b,
                              "g": gi})
                for k, (i_, base, wdt, entry) in enumerate(ct):
                    tile_info.append({"i": i_, "b": b, "base": base,
                                      "wdt": wdt, "call": cid, "tloc": k})
                    entries.append(entry)

    NT = len(tile_info)
    SLOTS = NT * 128

    soff = 0
    group_s0 = [None] * len(groups)
    group_send = [0] * len(groups)
    for t, ti in enumerate(tile_info):
        gi = calls[ti["call"]]["g"]
        if group_s0[gi] is None:
            group_s0[gi] = soff
        ti["soff"] = soff
        soff += ti["wdt"]
        group_send[gi] = soff
    SUMW = soff

    idx16 = np.zeros((NCORE, SLOTS), np.int16)
    sdata = np.zeros((NCORE, 128, SUMW), np.float16)
    for t, (ti, entry) in enumerate(zip(tile_info, entries)):
        s0 = t * 128
        for c in range(NCORE):
            cols, slocs, ws = entry[c]
            n = len(cols)
            if n == 0:
                continue
            idx16[c, s0:s0 + n] = slocs.astype(np.int16)
            sdata[c, np.arange(n), ti["soff"] + (cols - ti["base"])] = (
                ws.astype(np.float16))

    idx_sb = np.zeros((NCORE, 128, SLOTS // 16), np.int16)
    off16 = 0
    for call in calls:
        call["idx_off16"] = off16
        nsl = call["nt"] * 128
        s0 = call["t0"] * 128
        for c in range(NCORE):
            seg = idx16[c, s0:s0 + nsl]
            idx_sb[c, :, off16:off16 + nsl // 16] = np.tile(
                seg.reshape(nsl // 16, 16).T, (8, 1))
        off16 += nsl // 16

    static = {"groups": groups, "tile_info": tile_info, "calls": calls,
              "NT": NT, "SLOTS": SLOTS, "SUMW": SUMW,
              "group_s0": group_s0, "group_send": group_send}
    return static, {"idx_sb": idx_sb, "sdata": sdata, "idx16": idx16}


def _install_ntff_hook():
    from concourse import bass_utils
    try:
        import antenv
        from trn_agent_boot.trn_boot import _ntff_profile_via_ctypes
    except Exception:
        return
    if "antenv.axon_hooks" in sys.modules:
        return
    mod = types.ModuleType("antenv.axon_hooks")
    state = {"hook": None}
    mod.set_axon_ntff_profile_hook = lambda h: state.__setitem__("hook", h)
    mod.get_axon_ntff_profile_hook = lambda: state["hook"]
    sys.modules["antenv.axon_hooks"] = mod
    antenv.axon_hooks = mod
    mod.set_axon_ntff_profile_hook(
        _ntff_profile_via_ctypes("/opt/axon/libaxon_pjrt.so"))
    bass_utils.upload_artifacts = lambda tmpdir: f"local:{tmpdir}"


def _build(static):
    import concourse.bass as bass
    import concourse.bacc as bacc
    import concourse.tile as tile
    import concourse.mybir as mybir
    from concourse.masks import make_identity

    f32 = mybir.dt.float32
    f16 = mybir.dt.float16
    i16 = mybir.dt.int16
    AF = mybir.ActivationFunctionType
    OP = mybir.AluOpType
    AX = mybir.AxisListType

    groups = static["groups"]
    tile_info = static["tile_info"]
    calls = static["calls"]
    SLOTS = static["SLOTS"]
    SUMW = static["SUMW"]
    group_s0 = static["group_s0"]
    group_send = static["group_send"]

    # per-chunk tile lists (slot order)
    chunk_tiles = {i: [] for i in range(NCHK)}
    for t, ti in enumerate(tile_info):
        chunk_tiles[ti["i"]].append(t)

    nc = bacc.Bacc("TRN2", target_bir_lowering=False, debug=False,
                   num_devices=NCORE, num_swdge_queues=4)

    x_d = nc.dram_tensor("x_sh", [NPC, F_IN], f32, kind="ExternalInput").ap()
    W1_d = nc.dram_tensor("w1", [F_IN, HID], f32, kind="ExternalInput").ap()
    W2_d = nc.dram_tensor("w2", [HID, CLS], f32, kind="ExternalInput").ap()
    b1_d = nc.dram_tensor("b1c", [128, 2], f32, kind="ExternalInput").ap()
    b2_d = nc.dram_tensor("b2r", [128, CLS], f32, kind="ExternalInput").ap()
    idx_d = nc.dram_tensor("idxs", [128, SLOTS // 16], i16,
                           kind="ExternalInput").ap()
    sd_d = nc.dram_tensor("sdata", [128, SUMW], f16, kind="ExternalInput").ap()
    out_d = nc.dram_tensor("out", [NPC, CLS], f32, kind="ExternalOutput").ap()

    agin = nc.dram_tensor("agin", [NPC, 128], f16).ap()
    ztab = nc.dram_tensor("ztab", [N, 128], f16, addr_space="Shared").ap()

    with tile.TileContext(nc) as tc:
        with (
            tc.tile_pool(name="const", bufs=1) as cp,
            tc.tile_pool(name="resident", bufs=1) as rp,
            tc.tile_pool(name="mlp", bufs=3) as mp,
            tc.tile_pool(name="gb", bufs=5) as gp,
            tc.tile_pool(name="st", bufs=2) as stp,
            tc.tile_pool(name="sm", bufs=3) as smp,
        ):
            # constants / residents
            idx_t = rp.tile([128, SLOTS // 16], i16)
            nc.sync.dma_start(idx_t[:], idx_d[:])
            W1_t = cp.tile([128, HID], f32)
            nc.sync.dma_start(W1_t[:], W1_d[:])
            W2a_t = cp.tile([128, CLS], f32)
            nc.sync.dma_start(W2a_t[:], W2_d[0:128, :])
            W2b_t = cp.tile([128, CLS], f32)
            nc.sync.dma_start(W2b_t[:], W2_d[128:256, :])
            b1_t = cp.tile([128, 2], f32)
            nc.sync.dma_start(b1_t[:], b1_d[:])
            b2_t = cp.tile([128, CLS], f32)
            nc.sync.dma_start(b2_t[:], b2_d[:])
            ident = cp.tile([128, 128], f32)
            make_identity(nc, ident[:])
            zeroS = cp.tile([128, 128], f16)
            nc.vector.memset(zeroS[:], 0.0)
            ah_t = rp.tile([128, NCHK * 64], f32)
            nc.vector.memset(ah_t[:], 0.0)
            stg = rp.tile([128, NCHK * 128], f16)
            nc.vector.memset(stg[:], 0.0)

            # ---- MLP: z0 = relu(x@W1+b1)@W2+b2 ----
            with tc.tile_pool(name="psmlp", bufs=2, space="PSUM") as pmp:
                for i in range(NCHK):
                    sz = _chunk_size(i)
                    xt = mp.tile([128, F_IN], f32, tag="xt")
                    nc.sync.dma_start(xt[0:sz, :], x_d[CH * i:CH * i + sz, :])
                    pxT = pmp.tile([128, 128], f32, tag="pmlp")
                    nc.tensor.transpose(pxT[:, 0:sz], xt[0:sz, :],
                                        ident[0:sz, 0:sz])
                    xT = mp.tile([128, 128], f32, tag="xT")
                    nc.scalar.activation(xT[:, 0:sz], pxT[:, 0:sz], AF.Copy)
                    relus = []
                    for h in range(2):
                        ph = pmp.tile([128, 128], f32, tag="pmlp")
                        nc.tensor.matmul(ph[:, 0:sz],
                                         lhsT=W1_t[:, 128 * h:128 * (h + 1)],
                                         rhs=xT[:, 0:sz], start=True,
                                         stop=True)
                        rh = mp.tile([128, 128], f32, tag=f"relu{h}")
                        nc.scalar.activation(rh[:, 0:sz], ph[:, 0:sz],
                                             AF.Relu, bias=b1_t[:, h:h + 1])
                        relus.append(rh)
                    pz = pmp.tile([128, 128], f32, tag="pmlp")
                    for h in range(2):
                        nc.tensor.matmul(pz[0:sz, 0:CLS],
                                         lhsT=relus[h][:, 0:sz],
                                         rhs=(W2a_t if h == 0 else W2b_t)[:],
                                         start=(h == 0), stop=(h == 1))
                    z0 = mp.tile([128, CLS], f32, tag="z0")
                    nc.vector.tensor_tensor(out=z0[0:sz, :],
                                            in0=pz[0:sz, 0:CLS],
                                            in1=b2_t[0:sz, :], op=OP.add)
                    nc.vector.tensor_copy(
                        out=stg[0:sz, 128 * i:128 * i + CLS], in_=z0[0:sz, :])
                    nc.scalar.mul(ah_t[0:sz, 64 * i:64 * i + CLS],
                                  z0[0:sz, :], ALPHA)

            stg3 = stg[:].rearrange("p (i f) -> p i f", f=128)
            ag_dst1 = agin[0:(NCHK - 1) * CH, :].rearrange(
                "(i p) f -> p i f", p=128)

            def do_ag():
                nc.sync.dma_start(ag_dst1[:], stg3[:, 0:NCHK - 1, :])
                nc.sync.dma_start(agin[(NCHK - 1) * CH:NPC, :],
                                  stg[0:_chunk_size(NCHK - 1),
                                      128 * (NCHK - 1):128 * NCHK])
                nc.gpsimd.collective_compute(
                    "AllGather", mybir.AluOpType.bypass,
                    replica_groups=[list(range(NCORE))],
                    ins=[agin[:].opt()], outs=[ztab[:].opt()])

            do_ag()

            # ---- K propagation steps ----
            with tc.tile_pool(name="pschunk", bufs=8,
                              space="PSUM") as psp:
                calls_of_group = {}
                for cid, call in enumerate(calls):
                    calls_of_group.setdefault(call["g"], []).append(cid)

                max_sw = max(group_send[g] - group_s0[g]
                             for g in range(len(groups)))
                for k in range(1, K_ITERS + 1):
                    for gi, grp in enumerate(groups):
                        sw = group_send[gi] - group_s0[gi]
                        st_g = stp.tile([128, max_sw], f16, tag="stg")
                        nc.sync.dma_start(
                            st_g[:, 0:sw],
                            sd_d[:, group_s0[gi]:group_send[gi]])
                        gtile = {}
                        for qi, cid in enumerate(calls_of_group[gi]):
                            call = calls[cid]
                            nt = call["nt"]
                            b = call["b"]
                            g = gp.tile([128, MAX_CALL_TILES, 128], f16, tag="g")
                            nc.gpsimd.dma_gather(
                                g[:, 0:nt, :],
                                ztab[BLK * b:BLK * (b + 1), :],
                                idx_t[:, call["idx_off16"]:
                                      call["idx_off16"] + nt * 8],
                                nt * 128, nt * 128, 128,
                                single_packet=True,
                                queue_num=qi % 4,
                            )
                            gtile[cid] = g
                        for i in grp:
                            sz = _chunk_size(i)
                            ps = psp.tile([128, 64], f32, tag="ps")
                            nc.tensor.matmul(ps[:, :], lhsT=zeroS[:, 0:128],
                                             rhs=zeroS[:, 0:64],
                                             start=True, stop=False)
                            tl = chunk_tiles[i]
                            for j, t in enumerate(tl):
                                ti = tile_info[t]
                                loff = ti["soff"] - group_s0[gi]
                                nc.tensor.matmul(
                                    ps[ti["base"]:ti["base"] + ti["wdt"], 0:64],
                                    lhsT=st_g[:, loff:loff + ti["wdt"]],
                                    rhs=gtile[ti["call"]][:, ti["tloc"], 0:64],
                                    start=False, stop=(j == len(tl) - 1))
                            if k < K_ITERS:
                                nc.vector.tensor_tensor(
                                    out=stg[0:sz, 128 * i:128 * i + 64],
                                    in0=ps[0:sz, 0:64],
                                    in1=ah_t[0:sz, 64 * i:64 * (i + 1)],
                                    op=OP.add)
                            else:
                                zf = smp.tile([128, 64], f32, tag="zf")
                                nc.vector.tensor_tensor(
                                    out=zf[0:sz, :], in0=ps[0:sz, 0:64],
                                    in1=ah_t[0:sz, 64 * i:64 * (i + 1)],
                                    op=OP.add)
                                m = smp.tile([128, 1], f32, tag="m")
                                nc.vector.tensor_reduce(
                                    m[0:sz, :], zf[0:sz, 0:CLS], axis=AX.X,
                                    op=OP.max)
                                nm = smp.tile([128, 1], f32, tag="nm")
                                nc.vector.tensor_scalar_mul(
                                    nm[0:sz, :], m[0:sz, :], -1.0)
                                e = smp.tile([128, CLS], f32, tag="e")
                                nc.scalar.activation(e[0:sz, :], zf[0:sz, 0:CLS],
                                                     AF.Exp, bias=nm[0:sz, 0:1])
                                s = smp.tile([128, 1], f32, tag="s")
                                nc.vector.tensor_reduce(
                                    s[0:sz, :], e[0:sz, :], axis=AX.X, op=OP.add)
                                ls = smp.tile([128, 1], f32, tag="ls")
                                nc.scalar.activation(ls[0:sz, :], s[0:sz, :],
                                                     AF.Ln)
                                offs = smp.tile([128, 1], f32, tag="offs")
                                nc.vector.tensor_tensor(
                                    out=offs[0:sz, :], in0=m[0:sz, :],
                                    in1=ls[0:sz, :], op=OP.add)
                                res = smp.tile([128, CLS], f32, tag="res")
                                nc.vector.tensor_scalar(
                                    out=res[0:sz, :], in0=zf[0:sz, 0:CLS],
                                    scalar1=offs[0:sz, 0:1], scalar2=None,
                                    op0=OP.subtract)
                                nc.sync.dma_start(
                                    out_d[CH * i:CH * i + sz, :], res[0:sz, :])
                    if k < K_ITERS:
                        do_ag()

    nc.compile()
    return nc


_CACHE = {}


def kernel(x, edge_index, W1, b1, W2, b2):
    global LAST_EXEC_NS, LAST_SCOPES
    from concourse import bass_utils

    x = np.asarray(x, np.float32)
    ei = np.asarray(edge_index)
    W1 = np.asarray(W1, np.float32)
    b1 = np.asarray(b1, np.float32)
    W2 = np.asarray(W2, np.float32)
    b2 = np.asarray(b2, np.float32)

    static, per_core = _preprocess(ei)
    nc = _build(static)

    b1c = np.stack([b1[0:128], b1[128:256]], axis=1).astype(np.float32)
    b1c = np.ascontiguousarray(b1c)
    b2r = np.ascontiguousarray(np.tile(b2[None, :], (128, 1)).astype(np.float32))

    in_maps = []
    for c in range(NCORE):
        in_maps.append({
            "x_sh": np.ascontiguousarray(x[c * NPC:(c + 1) * NPC]),
            "w1": W1, "w2": W2, "b1c": b1c, "b2r": b2r,
            "idxs": np.ascontiguousarray(per_core["idx_sb"][c]),
            "sdata": np.ascontiguousarray(per_core["sdata"][c]),
        })

    if TRACE:
        _install_ntff_hook()
    res = bass_utils.run_bass_kernel_spmd(
        nc, in_maps, core_ids=list(range(NCORE)), trace=TRACE)
    LAST_EXEC_NS = res.exec_time_ns
    LAST_SCOPES = res.per_core_scope_times

    out = np.concatenate([res.results[c]["out"] for c in range(NCORE)], axis=0)
    return out.astype(np.float32)



# revision 6
# speedup vs baseline: 1.3625x; 1.3416x over previous
"""APPNP GNN forward on 8 Trainium2 NeuronCores (Bass/Tile, SPMD).

Strategy (all 8 cores share one instruction stream; per-core data differs):
  - nodes sharded 12500/core; MLP data-parallel; z staged fp16 [128, NCHK*64]
    in SBUF, mirrored to HBM as [128, NCHK, 128] (256B rows) and replicated
    via AllGather -> ztab [8*12544, 128] each step
  - edges (no self-loops) partitioned by dst core, grouped by (dst chunk of
    128, src block = 2 core-images), packed into 128-slot tiles; slots
    gathered via dma_gather (int16 block-local row ids, 256B rows), calls
    round-robined over 4 SWDGE queues
  - per tile, a [128, 128] fp16 S matrix is built ON-CHIP by one DVE
    tensor_scalar (iota == dcol) * w; aggregation via TensorE into a
    per-chunk PSUM [128, 64] initialized with alpha*h via an identity
    matmul; self-loops applied as an on-chip diag(selfw) matmul against the
    local previous-z staging
  - final iteration computes log_softmax on-chip from PSUM, fp32 out
"""
import sys, os, types

sys.path.insert(0, "/opt/trn_rl_repo")
import numpy as np

N = 100000
NCORE = 8
NPC = N // NCORE
CH = 128
NCHK = (NPC + CH - 1) // CH  # 98
ROWS_PC = CH * NCHK          # 12544 rows per core image
BLOCKS = 4
BLK_R = 2 * ROWS_PC          # 25088 rows per block (2 core images)
GSIZE = 8
ALPHA = 0.1
MAX_CALL_TILES = 16
K_ITERS = 10
F_IN = 128
HID = 256
CLS = 47

TRACE = False           # set by test harness for NTFF profiling
LAST_EXEC_NS = None
LAST_SCOPES = None


def _chunk_size(i):
    return min(CH, NPC - CH * i)


def _preprocess(edge_index):
    src = np.asarray(edge_index[0], dtype=np.int64)
    dst = np.asarray(edge_index[1], dtype=np.int64)
    E = src.shape[0]

    deg = np.bincount(dst, minlength=N).astype(np.float64) + 1.0
    dinv = 1.0 / np.sqrt(deg)
    ew = (dinv[src] * dinv[dst]) * (1.0 - ALPHA)
    selfw = ((dinv * dinv) * (1.0 - ALPHA)).astype(np.float32)

    core_d = dst // NPC
    dloc = dst % NPC
    chunk = dloc // CH
    dcol = dloc % CH
    s_core = src // NPC
    sl = src % NPC
    rowloc = (s_core % 2) * ROWS_PC + (sl % CH) * NCHK + (sl // CH)
    blk = s_core // 2

    order = np.lexsort((rowloc, blk, chunk, core_d))
    core_s = core_d[order]; chunk_s = chunk[order]; blk_s = blk[order]
    dcol_s = dcol[order]; rowloc_s = rowloc[order]
    w_s = ew[order].astype(np.float32)

    key = ((core_s * NCHK) + chunk_s) * BLOCKS + blk_s
    nkeys = NCORE * NCHK * BLOCKS
    starts = np.searchsorted(key, np.arange(nkeys), side="left")
    ends = np.searchsorted(key, np.arange(nkeys), side="right")

    def cnt(c, i, b):
        k = (c * NCHK + i) * BLOCKS + b
        return ends[k] - starts[k]

    def run(c, i, b):
        k = (c * NCHK + i) * BLOCKS + b
        sl_ = slice(starts[k], ends[k])
        return dcol_s[sl_], rowloc_s[sl_], w_s[sl_]

    # tiles per (i, b): shared across cores (SPMD)
    nt_ib = np.zeros((NCHK, BLOCKS), np.int64)
    for i in range(NCHK):
        for b in range(BLOCKS):
            m = max(cnt(c, i, b) for c in range(NCORE))
            nt_ib[i, b] = -(-m // CH)

    groups = []
    i = 0
    while i < NCHK:
        groups.append(list(range(i, min(i + GSIZE, NCHK))))
        i += GSIZE

    # calls: per (group, b), tiles chunk-major, split by MAX_CALL_TILES;
    # issue order interleaves b across the call index for queue spread.
    tile_info = []
    calls = []
    calls_of_group = {}
    chunk_tiles = {i_: [] for i_ in range(NCHK)}
    for gi, g in enumerate(groups):
        percall = {}
        for b in range(BLOCKS):
            tlist = []
            for i_ in g:
                for k in range(int(nt_ib[i_, b])):
                    tlist.append((i_, b, k))
            percall[b] = [tlist[s:s + MAX_CALL_TILES]
                          for s in range(0, len(tlist), MAX_CALL_TILES)]
        order_calls = []
        ci = 0
        while True:
            any_ = False
            for b in range(BLOCKS):
                if ci < len(percall[b]):
                    order_calls.append(percall[b][ci])
                    any_ = True
            if not any_:
                break
            ci += 1
        cg = []
        for ct in order_calls:
            cid = len(calls)
            b = ct[0][1]
            calls.append({"t0": len(tile_info), "nt": len(ct), "b": b,
                          "g": gi})
            for k, (i_, b_, _) in enumerate(ct):
                t = len(tile_info)
                tile_info.append({"i": i_, "b": b_, "call": cid, "tloc": k})
                chunk_tiles[i_].append(t)
            cg.append(cid)
        calls_of_group[gi] = cg

    NT = len(tile_info)
    SLOTS = NT * CH

    # per-core slot fills
    idx16 = np.zeros((NCORE, SLOTS), np.int16)
    dcol_f = np.zeros((NCORE, CH, NT), np.float32)
    w_f = np.zeros((NCORE, CH, NT), np.float32)
    ptr = {}
    for c in range(NCORE):
        for i_ in range(NCHK):
            for b in range(BLOCKS):
                ptr[(c, i_, b)] = 0
    for t, ti in enumerate(tile_info):
        i_, b = ti["i"], ti["b"]
        s0 = t * CH
        for c in range(NCORE):
            dcs, rls, ws = run(c, i_, b)
            p0 = ptr[(c, i_, b)]
            n = min(CH, len(dcs) - p0)
            if n <= 0:
                continue
            idx16[c, s0:s0 + n] = rls[p0:p0 + n].astype(np.int16)
            dcol_f[c, 0:n, t] = dcs[p0:p0 + n].astype(np.float32)
            w_f[c, 0:n, t] = ws[p0:p0 + n]
            ptr[(c, i_, b)] = p0 + n

    # idx replicated into the 16-partition wrapped layout, 8x across 128
    idx_sb = np.zeros((NCORE, CH, SLOTS // 16), np.int16)
    off16 = 0
    for call in calls:
        call["idx_off16"] = off16
        nsl = call["nt"] * CH
        s0 = call["t0"] * CH
        for c in range(NCORE):
            seg = idx16[c, s0:s0 + nsl]
            idx_sb[c, :, off16:off16 + nsl // 16] = np.tile(
                seg.reshape(nsl // 16, 16).T, (8, 1))
        off16 += nsl // 16

    selfw_sb = np.zeros((NCORE, CH, NCHK), np.float32)
    for c in range(NCORE):
        sw = selfw[c * NPC:(c + 1) * NPC]
        swp = np.zeros(ROWS_PC, np.float32)
        swp[:NPC] = sw
        selfw_sb[c] = swp.reshape(NCHK, CH).T

    static = {"groups": groups, "tile_info": tile_info, "calls": calls,
              "calls_of_group": calls_of_group, "chunk_tiles": chunk_tiles,
              "NT": NT, "SLOTS": SLOTS}
    per_core = {"idx_sb": idx_sb, "dcol_f": dcol_f, "w_f": w_f,
                "selfw_sb": selfw_sb}
    return static, per_core


def _install_ntff_hook():
    from concourse import bass_utils
    try:
        import antenv
        from trn_agent_boot.trn_boot import _ntff_profile_via_ctypes
    except Exception:
        return
    if "antenv.axon_hooks" in sys.modules:
        return
    mod = types.ModuleType("antenv.axon_hooks")
    state = {"hook": None}
    mod.set_axon_ntff_profile_hook = lambda h: state.__setitem__("hook", h)
    mod.get_axon_ntff_profile_hook = lambda: state["hook"]
    sys.modules["antenv.axon_hooks"] = mod
    antenv.axon_hooks = mod
    mod.set_axon_ntff_profile_hook(
        _ntff_profile_via_ctypes("/opt/axon/libaxon_pjrt.so"))
    bass_utils.upload_artifacts = lambda tmpdir: f"local:{tmpdir}"


def _build(static):
    import concourse.bass as bass
    import concourse.bacc as bacc
    import concourse.tile as tile
    import concourse.mybir as mybir
    from concourse.masks import make_identity

    f32 = mybir.dt.float32
    f16 = mybir.dt.float16
    i16 = mybir.dt.int16
    i32 = mybir.dt.int32
    AF = mybir.ActivationFunctionType
    OP = mybir.AluOpType
    AX = mybir.AxisListType

    groups = static["groups"]
    tile_info = static["tile_info"]
    calls = static["calls"]
    calls_of_group = static["calls_of_group"]
    chunk_tiles = static["chunk_tiles"]
    SLOTS = static["SLOTS"]
    NT = static["NT"]

    nc = bacc.Bacc("TRN2", target_bir_lowering=False, debug=False,
                   num_devices=NCORE, num_swdge_queues=4)

    x_d = nc.dram_tensor("x_sh", [NPC, F_IN], f32, kind="ExternalInput").ap()
    W1_d = nc.dram_tensor("w1", [F_IN, HID], f32, kind="ExternalInput").ap()
    W2_d = nc.dram_tensor("w2", [HID, CLS], f32, kind="ExternalInput").ap()
    b1_d = nc.dram_tensor("b1c", [128, 2], f32, kind="ExternalInput").ap()
    b2_d = nc.dram_tensor("b2r", [128, CLS], f32, kind="ExternalInput").ap()
    idx_d = nc.dram_tensor("idxs", [128, SLOTS // 16], i16,
                           kind="ExternalInput").ap()
    dcol_d = nc.dram_tensor("dcolf", [128, NT], f32,
                            kind="ExternalInput").ap()
    w_d = nc.dram_tensor("wf", [128, NT], f32, kind="ExternalInput").ap()
    selfw_d = nc.dram_tensor("selfwf", [128, NCHK], f32,
                             kind="ExternalInput").ap()
    out_d = nc.dram_tensor("out", [NPC, CLS], f32, kind="ExternalOutput").ap()

    hstage = nc.dram_tensor("hstage", [128, NCHK * 128], f16).ap()
    ztab = nc.dram_tensor("ztab", [NCORE * ROWS_PC, 128], f16,
                          addr_space="Shared").ap()

    with tile.TileContext(nc) as tc:
        with (
            tc.tile_pool(name="const", bufs=1) as cp,
            tc.tile_pool(name="resident", bufs=1) as rp,
            tc.tile_pool(name="mlp", bufs=3) as mp,
            tc.tile_pool(name="gb", bufs=16) as gp,
            tc.tile_pool(name="sb", bufs=8) as sp,
            tc.tile_pool(name="sm", bufs=3) as smp,
        ):
            # constants / residents
            idx_t = rp.tile([128, SLOTS // 16], i16)
            nc.sync.dma_start(idx_t[:], idx_d[:])
            dcol_t = rp.tile([128, NT], f32)
            nc.sync.dma_start(dcol_t[:], dcol_d[:])
            w_t = rp.tile([128, NT], f32)
            nc.sync.dma_start(w_t[:], w_d[:])
            selfw_t = rp.tile([128, NCHK], f32)
            nc.sync.dma_start(selfw_t[:], selfw_d[:])
            W1_t = cp.tile([128, HID], f32)
            nc.sync.dma_start(W1_t[:], W1_d[:])
            W2a_t = cp.tile([128, CLS], f32)
            nc.sync.dma_start(W2a_t[:], W2_d[0:128, :])
            W2b_t = cp.tile([128, CLS], f32)
            nc.sync.dma_start(W2b_t[:], W2_d[128:256, :])
            b1_t = cp.tile([128, 2], f32)
            nc.sync.dma_start(b1_t[:], b1_d[:])
            b2_t = cp.tile([128, CLS], f32)
            nc.sync.dma_start(b2_t[:], b2_d[:])
            ident = cp.tile([128, 128], f32)
            make_identity(nc, ident[:])
            ident16 = cp.tile([128, 128], f16)
            make_identity(nc, ident16[:])
            io32 = cp.tile([128, 128], i32)
            nc.gpsimd.iota(io32[:], pattern=[[1, 128]], base=0,
                           channel_multiplier=0)
            iota16 = cp.tile([128, 128], f16)
            nc.vector.tensor_scalar(out=iota16[:], in0=io32[:], scalar1=0.0,
                                    scalar2=None, op0=OP.add)
            ip32 = cp.tile([128, 1], i32)
            nc.gpsimd.iota(ip32[:], pattern=[[0, 1]], base=0,
                           channel_multiplier=1)
            pcol = cp.tile([128, 1], f32)
            nc.vector.tensor_scalar(out=pcol[:], in0=ip32[:], scalar1=0.0,
                                    scalar2=None, op0=OP.add)

            ah_t = rp.tile([128, NCHK * 64], f16)
            nc.vector.memset(ah_t[:], 0.0)
            stgA = rp.tile([128, NCHK * 64], f16)
            nc.vector.memset(stgA[:], 0.0)
            stgB = rp.tile([128, NCHK * 64], f16)
            nc.vector.memset(stgB[:], 0.0)

            # ---- MLP: z0 = relu(x@W1+b1)@W2+b2 ----
            with tc.tile_pool(name="psmlp", bufs=2, space="PSUM") as pmp:
                for i in range(NCHK):
                    sz = _chunk_size(i)
                    xt = mp.tile([128, F_IN], f32, tag="xt")
                    nc.sync.dma_start(xt[0:sz, :], x_d[CH * i:CH * i + sz, :])
                    pxT = pmp.tile([128, 128], f32, tag="pmlp")
                    nc.tensor.transpose(pxT[:, 0:sz], xt[0:sz, :],
                                        ident[0:sz, 0:sz])
                    xT = mp.tile([128, 128], f32, tag="xT")
                    nc.scalar.activation(xT[:, 0:sz], pxT[:, 0:sz], AF.Copy)
                    relus = []
                    for h in range(2):
                        ph = pmp.tile([128, 128], f32, tag="pmlp")
                        nc.tensor.matmul(ph[:, 0:sz],
                                         lhsT=W1_t[:, 128 * h:128 * (h + 1)],
                                         rhs=xT[:, 0:sz], start=True,
                                         stop=True)
                        rh = mp.tile([128, 128], f32, tag=f"relu{h}")
                        nc.scalar.activation(rh[:, 0:sz], ph[:, 0:sz],
                                             AF.Relu, bias=b1_t[:, h:h + 1])
                        relus.append(rh)
                    pz = pmp.tile([128, 128], f32, tag="pmlp")
                    for h in range(2):
                        nc.tensor.matmul(pz[0:sz, 0:CLS],
                                         lhsT=relus[h][:, 0:sz],
                                         rhs=(W2a_t if h == 0 else W2b_t)[:],
                                         start=(h == 0), stop=(h == 1))
                    z0 = mp.tile([128, CLS], f32, tag="z0")
                    nc.vector.tensor_tensor(out=z0[0:sz, :],
                                            in0=pz[0:sz, 0:CLS],
                                            in1=b2_t[0:sz, :], op=OP.add)
                    nc.vector.tensor_copy(
                        out=stgA[0:sz, 64 * i:64 * i + CLS], in_=z0[0:sz, :])
                    nc.scalar.mul(ah_t[0:sz, 64 * i:64 * i + CLS],
                                  z0[0:sz, :], ALPHA)

            hview = hstage[:].rearrange("p (i f) -> p i f", f=128)

            def stage_groups(stg, gis):
                stg3 = stg[:].rearrange("p (i f) -> p i f", f=64)
                for gi in gis:
                    g = groups[gi]
                    nc.sync.dma_start(hview[:, g[0]:g[-1] + 1, 0:64],
                                      stg3[:, g[0]:g[-1] + 1, :])

            def do_ag():
                nc.gpsimd.collective_compute(
                    "AllGather", mybir.AluOpType.bypass,
                    replica_groups=[list(range(NCORE))],
                    ins=[hstage[:].opt()], outs=[ztab[:].opt()])

            stage_groups(stgA, range(len(groups)))
            do_ag()

            # ---- K propagation steps ----
            stg_prev, stg_new = stgA, stgB
            rr = 0
            with tc.tile_pool(name="pschunk", bufs=8, space="PSUM") as psp:
                for k in range(1, K_ITERS + 1):
                    for gi, grp in enumerate(groups):
                        gtile = {}
                        for cid in calls_of_group[gi]:
                            call = calls[cid]
                            nt = call["nt"]
                            b = call["b"]
                            g = gp.tile([128, MAX_CALL_TILES, 128], f16,
                                        tag="g")
                            nc.gpsimd.dma_gather(
                                g[:, 0:nt, :],
                                ztab[BLK_R * b:BLK_R * (b + 1), :],
                                idx_t[:, call["idx_off16"]:
                                      call["idx_off16"] + nt * 8],
                                nt * 128, nt * 128, 128,
                                single_packet=False,
                                queue_num=rr % 4,
                            )
                            rr += 1
                            gtile[cid] = g
                        for i in grp:
                            sz = _chunk_size(i)
                            ps = psp.tile([128, 64], f32, tag="ps")
                            nc.tensor.matmul(ps[:, :], lhsT=ident16[:],
                                             rhs=ah_t[:, 64 * i:64 * (i + 1)],
                                             start=True, stop=False)
                            sd = sp.tile([128, 128], f16, tag="sd")
                            nc.vector.tensor_scalar(
                                out=sd[:], in0=iota16[:], scalar1=pcol[:],
                                scalar2=selfw_t[:, i:i + 1],
                                op0=OP.is_equal, op1=OP.mult)
                            nc.tensor.matmul(
                                ps[:, :], lhsT=sd[:],
                                rhs=stg_prev[:, 64 * i:64 * (i + 1)],
                                start=False, stop=False)
                            tl = chunk_tiles[i]
                            for j, t in enumerate(tl):
                                ti = tile_info[t]
                                S = sp.tile([128, 128], f16, tag="S")
                                nc.vector.tensor_scalar(
                                    out=S[:], in0=iota16[:],
                                    scalar1=dcol_t[:, t:t + 1],
                                    scalar2=w_t[:, t:t + 1],
                                    op0=OP.is_equal, op1=OP.mult)
                                nc.tensor.matmul(
                                    ps[:, :], lhsT=S[:],
                                    rhs=gtile[ti["call"]][:, ti["tloc"], 0:64],
                                    start=False, stop=(j == len(tl) - 1))
                            if k < K_ITERS:
                                nc.scalar.activation(
                                    stg_new[0:sz, 64 * i:64 * i + 64],
                                    ps[0:sz, 0:64], AF.Copy)
                            else:
                                m = smp.tile([128, 1], f32, tag="m")
                                nc.vector.tensor_reduce(
                                    m[0:sz, :], ps[0:sz, 0:CLS], axis=AX.X,
                                    op=OP.max)
                                nm = smp.tile([128, 1], f32, tag="nm")
                                nc.vector.tensor_scalar_mul(
                                    nm[0:sz, :], m[0:sz, :], -1.0)
                                e = smp.tile([128, CLS], f32, tag="e")
                                nc.scalar.activation(e[0:sz, :],
                                                     ps[0:sz, 0:CLS],
                                                     AF.Exp,
                                                     bias=nm[0:sz, 0:1])
                                s = smp.tile([128, 1], f32, tag="s")
                                nc.vector.tensor_reduce(
                                    s[0:sz, :], e[0:sz, :], axis=AX.X,
                                    op=OP.add)
                                ls = smp.tile([128, 1], f32, tag="ls")
                                nc.scalar.activation(ls[0:sz, :], s[0:sz, :],
                                                     AF.Ln)
                                offs = smp.tile([128, 1], f32, tag="offs")
                                nc.vector.tensor_tensor(
                                    out=offs[0:sz, :], in0=m[0:sz, :],
                                    in1=ls[0:sz, :], op=OP.add)
                                res = smp.tile([128, CLS], f32, tag="res")
                                nc.vector.tensor_scalar(
                                    out=res[0:sz, :], in0=ps[0:sz, 0:CLS],
                                    scalar1=offs[0:sz, 0:1], scalar2=None,
                                    op0=OP.subtract)
                                nc.sync.dma_start(
                                    out_d[CH * i:CH * i + sz, :],
                                    res[0:sz, :])
                        if k < K_ITERS:
                            stage_groups(stg_new, [gi])
                    if k < K_ITERS:
                        do_ag()
                        stg_prev, stg_new = stg_new, stg_prev

    nc.compile()
    return nc


def kernel(x, edge_index, W1, b1, W2, b2):
    global LAST_EXEC_NS, LAST_SCOPES
    from concourse import bass_utils

    x = np.asarray(x, np.float32)
    ei = np.asarray(edge_index)
    W1 = np.asarray(W1, np.float32)
    b1 = np.asarray(b1, np.float32)
    W2 = np.asarray(W2, np.float32)
    b2 = np.asarray(b2, np.float32)

    static, per_core = _preprocess(ei)
    nc = _build(static)

    b1c = np.stack([b1[0:128], b1[128:256]], axis=1).astype(np.float32)
    b1c = np.ascontiguousarray(b1c)
    b2r = np.ascontiguousarray(np.tile(b2[None, :], (128, 1)).astype(np.float32))

    in_maps = []
    for c in range(NCORE):
        in_maps.append({
            "x_sh": np.ascontiguousarray(x[c * NPC:(c + 1) * NPC]),
            "w1": W1, "w2": W2, "b1c": b1c, "b2r": b2r,
            "idxs": np.ascontiguousarray(per_core["idx_sb"][c]),
            "dcolf": np.ascontiguousarray(per_core["dcol_f"][c]),
            "wf": np.ascontiguousarray(per_core["w_f"][c]),
            "selfwf": np.ascontiguousarray(per_core["selfw_sb"][c]),
        })

    if TRACE:
        _install_ntff_hook()
    res = bass_utils.run_bass_kernel_spmd(
        nc, in_maps, core_ids=list(range(NCORE)), trace=TRACE)
    LAST_EXEC_NS = res.exec_time_ns
    LAST_SCOPES = res.per_core_scope_times

    out = np.concatenate([res.results[c]["out"] for c in range(NCORE)], axis=0)
    return out.astype(np.float32)
